# revision 21
# baseline (speedup 1.0000x reference)
"""MixHop Trainium2 kernel v3 — diffuse raw X, then apply W.

Identity used: a@(X@Wz) == (a@X)@Wz (diffusion acts on nodes, W on
features), so the 1536-col pass A of v2 becomes a 768-col diffusion of
the raw feature block X, halving the dominant matmul:
    Y1  = ac@Xq                   (pass A'', feature-major psum out)
    z1  = lrelu(Y1q@W1z + v1)     (big 512-col matmuls, bias per-partition)
    wc  = Y1q@W2z4 (node-major)   (DoubleRow, 48 matmuls)
    z2  = lrelu(ac@wc + B + rc x v2)   (pass B, node-major, as v2)
    p0  = lrelu(Xq@W0z)           (from host feature-major X)
All rank-1 common-mode corrections stay host-exact (centered adjacency).
"""

import os
import sys

if "/opt/trn_rl_repo" not in sys.path:
    sys.path.insert(0, "/opt/trn_rl_repo")

import ml_dtypes
import numpy as np

import concourse.bass as bass
import concourse.tile as tile
from concourse import bacc, mybir
from concourse.bass_utils import run_bass_kernel_spmd

F = 64
O = 64
N = 2048
T = 12
NB = N // 128
KB = N // 256
C = O * T          # 768
XC = F * T         # 768 X columns, (t, f)
NQ = 3             # t-quads

F32 = mybir.dt.float32
BF16 = mybir.dt.bfloat16
FP8 = mybir.dt.float8e4
DR = mybir.MatmulPerfMode.DoubleRow
ACT_FUNC = mybir.ActivationFunctionType.Lrelu
LEAKY_SLOPE = 0.01


def build_nc(num_devices=8):
    nc = bacc.Bacc("TRN2", target_bir_lowering=False, debug=False,
                   num_devices=num_devices)

    # ---- DRAM I/O ----------------------------------------------------------
    # node-major X pairs (pass A'' stationary): [kb, p, j, (t,f)]
    xq_d = nc.dram_tensor("xq", [KB, 128, 2, XC], FP8, kind="ExternalInput").ap()
    # feature-major X quads (p0 moving): [q, (t2,f), t1, n]
    xtq_d = nc.dram_tensor("xtq", [NQ, 128, 2, N], FP8, kind="ExternalInput").ap()
    # moving adjacency for A'': [kb, p(m_low), j, n] = ac[n, m]
    adjm_d = nc.dram_tensor("adjm", [KB, 128, 2, N], FP8, kind="ExternalInput").ap()
    # stationary adjacency for pass B: [nb, p, kb, j, nl] = ac[nb*128+nl, m]
    adjc_d = nc.dram_tensor("adjc", [NB, 128, KB, 2, 128], FP8,
                            kind="ExternalInput").ap()
    # weight blocks (fp8): w1z/w0z [(t2,f), (t2,o)]; wz4 [(t2,f), t1, (u,o)]
    w1z_d = nc.dram_tensor("w1z", [128, 128], FP8, kind="ExternalInput").ap()
    w0z_d = nc.dram_tensor("w0z", [128, 128], FP8, kind="ExternalInput").ap()
    wz4_d = nc.dram_tensor("wz4", [128, 2, 256], FP8, kind="ExternalInput").ap()
    # v1col [ (t2,o), (q,t1) ]; vrow [v1|v2|B] replicated; rcol
    v1col_d = nc.dram_tensor("v1col", [128, 2 * NQ], F32, kind="ExternalInput").ap()
    vrow_d = nc.dram_tensor("vrow", [128, 3 * C], F32, kind="ExternalInput").ap()
    rcol_d = nc.dram_tensor("rcol", [128, NB], F32, kind="ExternalInput").ap()

    # outputs: p0/z1 feature-major [ (q,t1), (t2,o), n ]; z2 node-major
    p0t_d = nc.dram_tensor("p0t", [2 * NQ, 128, N], BF16, kind="ExternalOutput").ap()
    z1t_d = nc.dram_tensor("z1t", [2 * NQ, 128, N], BF16, kind="ExternalOutput").ap()
    z2_d = nc.dram_tensor("z2", [N, C], BF16, kind="ExternalOutput").ap()

    lrelu = ACT_FUNC
    add = mybir.AluOpType.add
    mult = mybir.AluOpType.mult
    amax = mybir.AluOpType.max

    with tile.TileContext(nc) as tc:
        with (
            tc.tile_pool(name="consts", bufs=1) as consts,
            tc.tile_pool(name="xq", bufs=1) as xqp,
            tc.tile_pool(name="adjm", bufs=1) as adjmp,
            tc.tile_pool(name="adjc", bufs=1) as adjcp,
            tc.tile_pool(name="y1t", bufs=1) as y1tp,
            tc.tile_pool(name="xtq", bufs=1) as xtqp,
            tc.tile_pool(name="wq", bufs=KB) as wqp,
            tc.tile_pool(name="zst", bufs=6) as zstp,
            tc.tile_pool(name="tmp", bufs=4) as tmpp,
        ):
            # ---- loads (sync ring), in need-order -------------------------
            xq_t = xqp.tile([128, KB * 2 * XC], FP8, name="xqall")
            nc.sync.dma_start(
                out=xq_t[:].rearrange("p (kb r) -> p kb r", kb=KB),
                in_=xq_d.rearrange("kb p j c -> p kb (j c)"),
            )
            adjm_ch = []
            for cix in range(4):
                amc = adjmp.tile([128, 2 * 2 * N], FP8, tag=f"adjm{cix}",
                                 name=f"adjm{cix}")
                nc.sync.dma_start(
                    out=amc[:].rearrange("p (kb r) -> p kb r", kb=2),
                    in_=adjm_d[2 * cix:2 * cix + 2].rearrange(
                        "kb p j n -> p kb (j n)"
                    ),
                )
                adjm_ch.append(amc)
            w1z_t = consts.tile([128, 128], FP8, tag="w1z")
            nc.sync.dma_start(out=w1z_t[:], in_=w1z_d)
            w0z_t = consts.tile([128, 128], FP8, tag="w0z")
            nc.sync.dma_start(out=w0z_t[:], in_=w0z_d)
            wz4_t = consts.tile([128, 512], FP8, tag="wz4")
            nc.sync.dma_start(
                out=wz4_t[:].rearrange("p (j c) -> p j c", j=2), in_=wz4_d
            )
            v1c_t = consts.tile([128, 2 * NQ], F32, tag="v1c")
            nc.sync.dma_start(out=v1c_t[:], in_=v1col_d)
            vrow_t = consts.tile([128, 3 * C], F32, tag="vrow")
            nc.sync.dma_start(out=vrow_t[:], in_=vrow_d)
            rc_t = consts.tile([128, NB], F32, tag="rc")
            nc.sync.dma_start(out=rc_t[:], in_=rcol_d)
            xtq_t = xtqp.tile([128, NQ * 2 * N], FP8, name="xtqall")
            nc.sync.dma_start(
                out=xtq_t[:].rearrange("p (q r) -> p q r", q=NQ),
                in_=xtq_d.rearrange("q p j n -> p q (j n)"),
            )
            adjc_t = adjcp.tile([128, NB * N], FP8, name="adjcall")
            nc.sync.dma_start(
                out=adjc_t[:].rearrange("p (nb r) -> p nb r", nb=NB),
                in_=adjc_d.rearrange("nb p a b c -> p nb (a b c)"),
            )

            def xq_sl(kb, ch):  # stationary [128, 2, 128] for A'' chunk ch
                return xq_t[:].rearrange(
                    "p (kb j c) -> p kb j c", kb=KB, j=2
                )[:, kb, :, ch * 128:(ch + 1) * 128]

            def adjm_sl(kb, ns):  # moving [128, 2, 256]
                return adjm_ch[kb // 2][:].rearrange(
                    "p (kb j n) -> p kb j n", kb=2, j=2
                )[:, kb % 2, :, ns * 256:(ns + 1) * 256]

            def adjc_sl(nb):  # pass-B stationary [128, kb, j, nl]
                return adjc_t[:, nb * N:(nb + 1) * N].rearrange(
                    "p (kb j nl) -> p kb j nl", kb=KB, j=2
                )

            # ---- pass A'': Y1T[ch] = (ac@X)^T chunk, feature-major --------
            # psum [c2=128, n=2048] (4 banks) x 2 bufs; 6 chunks (q, t1).
            y1 = []
            for q in range(NQ):
                y1.append(
                    y1tp.tile([128, 2 * N], FP8, tag=f"y1{q}", name=f"y1q{q}")
                )
            # kb-outer over chunk pairs (one t-quad per sweep): the first
            # sweep consumes adjm slabs as they stream in instead of
            # waiting for the full 4.2MB load.
            with tc.tile_pool(name="psA", bufs=2, space="PSUM") as psA:
                for q in range(NQ):
                    pzs = [
                        psA.tile([128, N], F32, tag="A", name=f"pzA{q}_{t1}")
                        for t1 in range(2)
                    ]
                    for kb in range(KB):
                        for t1 in range(2):
                            lhsT = xq_sl(kb, 2 * q + t1)
                            for ns in range(KB):
                                nc.tensor.matmul(
                                    pzs[t1][:, ns * 256:(ns + 1) * 256],
                                    lhsT,
                                    adjm_sl(kb, ns),
                                    start=(kb == 0 and ns % 2 == 0),
                                    stop=(kb == KB - 1 and ns % 2 == 1),
                                    perf_mode=DR,
                                )
                    for t1 in range(2):
                        dst = y1[q][:].rearrange("p (j n) -> p j n", j=2)[:, t1]
                        nc.vector.tensor_copy(dst[:, 0:N // 2],
                                              pzs[t1][:, 0:N // 2])
                        nc.scalar.activation(
                            dst[:, N // 2:N], pzs[t1][:, N // 2:N],
                            mybir.ActivationFunctionType.Copy,
                        )

            # ---- wc-Wmult: wc = Y1q@W2z4, node-major (for pass B) -----
            # two nb per psum bank, one batched 3D fp8 copy per nb-pair.
            wq = []
            for k in range(KB):
                wq.append(
                    wqp.tile([128, 2 * C], FP8, tag="wq", name=f"wq{k}")
                )
            with tc.tile_pool(name="psW", bufs=3, space="PSUM") as psW:
                for q in range(NQ):
                    y1v = y1[q][:].rearrange("p (j n) -> p j n", j=2)
                    for k in range(KB):
                        pw = psW.tile([128, 512], F32, tag="W")
                        for u in range(2):
                            nb = 2 * k + u
                            nc.tensor.matmul(
                                pw[:, u * 256:(u + 1) * 256],
                                y1v[:, :, nb * 128:(nb + 1) * 128],
                                wz4_t[:].rearrange("p (j c) -> p j c", j=2),
                                start=(u == 0), stop=(u == 1),
                                perf_mode=DR,
                            )
                        nc.vector.tensor_copy(
                            wq[k][:].rearrange("p (j c) -> p j c", j=2)[
                                :, :, q * 256:(q + 1) * 256
                            ],
                            pw[:].rearrange("p (u c) -> p u c", u=2),
                        )

                # ---- pass B with z1/p0 Wmults interleaved (3 per nb) ------
                # z1 drain: one ACT (bias=v1col + lrelu); p0 drain: one DVE
                # scalar_tensor_tensor leaky (max(0.01x, x)).
                wjobs = [("z1", ch, ns) for ch in range(2 * NQ) for ns in range(4)]
                wjobs += [("p0", ch, ns) for ch in range(2 * NQ) for ns in range(4)]

                def wmult_job(kind, ch, ns):
                    q, t1 = ch // 2, ch % 2
                    pw = psW.tile([128, 512], F32, tag="W")
                    if kind == "z1":
                        mv = y1[q][:].rearrange("p (j n) -> p j n", j=2)[
                            :, t1, ns * 512:(ns + 1) * 512
                        ]
                        nc.tensor.matmul(pw[:], w1z_t[:], mv,
                                         start=True, stop=True)
                        zt = zstp.tile([128, 512], BF16, tag="zst")
                        nc.scalar.activation(
                            zt[:], pw[:], lrelu, alpha=0.01,
                            bias=v1c_t[:, ch:ch + 1],
                        )
                        nc.scalar.dma_start(
                            out=z1t_d[ch, :, ns * 512:(ns + 1) * 512], in_=zt[:]
                        )
                    else:
                        mv = xtq_t[:].rearrange(
                            "p (q j n) -> p q j n", q=NQ, j=2
                        )[:, q, t1, ns * 512:(ns + 1) * 512]
                        nc.tensor.matmul(pw[:], w0z_t[:], mv,
                                         start=True, stop=True)
                        zt = zstp.tile([128, 512], BF16, tag="zst")
                        nc.scalar.activation(zt[:], pw[:], lrelu, alpha=0.01)
                        nc.scalar.dma_start(
                            out=p0t_d[ch, :, ns * 512:(ns + 1) * 512], in_=zt[:]
                        )

                with tc.tile_pool(name="psB", bufs=2, space="PSUM") as psB:
                    for nb in range(NB):
                        pz = psB.tile([128, C], F32, tag="B")
                        sv = adjc_sl(nb)
                        for kb in range(KB):
                            wv = wq[kb][:].rearrange("p (j c) -> p j c", j=2)
                            for s in range(3):
                                nc.tensor.matmul(
                                    pz[:, s * 256:(s + 1) * 256],
                                    sv[:, kb],
                                    wv[:, :, s * 256:(s + 1) * 256],
                                    start=(kb == 0 and s % 2 == 0),
                                    stop=(kb == KB - 1 and s >= 1),
                                    perf_mode=DR,
                                )
                        for _ in range(3):
                            if wjobs:
                                wmult_job(*wjobs.pop(0))
                        s2 = tmpp.tile([128, C], F32, tag="tmp")
                        nc.vector.scalar_tensor_tensor(
                            s2[:], vrow_t[:, C:2 * C], rc_t[:, nb:nb + 1],
                            vrow_t[:, 2 * C:3 * C], mult, add,
                        )
                        tmp = tmpp.tile([128, C], F32, tag="tmp")
                        nc.vector.tensor_tensor(tmp[:], pz[:], s2[:], add)
                        zt = zstp.tile([128, C], BF16, tag="zstB")
                        nc.vector.scalar_tensor_tensor(
                            zt[:], tmp[:], LEAKY_SLOPE, tmp[:], mult, amax
                        )
                        nc.scalar.dma_start(
                            out=z2_d[nb * 128:(nb + 1) * 128, :], in_=zt[:]
                        )
                    while wjobs:
                        wmult_job(*wjobs.pop(0))

    nc.finalize()
    return nc


def host_prep(x_b, adj_b, W0, W1, W2):
    Xf = np.ascontiguousarray(x_b.transpose(1, 2, 0)).reshape(N, XC)  # [n,(t,f)]
    X8 = Xf.astype(ml_dtypes.float8_e4m3fn)
    xq = np.ascontiguousarray(
        X8.reshape(KB, 2, 128, XC).transpose(0, 2, 1, 3)
    )
    xtq = np.ascontiguousarray(
        X8.reshape(N, NQ, 2, 2, F).transpose(1, 3, 4, 2, 0).reshape(NQ, 128, 2, N)
    )
    ac = (adj_b - np.float32(0.5))
    ac8 = ac.astype(ml_dtypes.float8_e4m3fn)
    acT = np.ascontiguousarray(ac8.T)  # [m, n]
    adjm = np.ascontiguousarray(
        acT.reshape(KB, 2, 128, N).transpose(0, 2, 1, 3)
    )
    adjc = np.ascontiguousarray(
        ac8.reshape(NB, 128, KB, 2, 128).transpose(0, 4, 2, 3, 1)
    )

    def blockdiag2(Wm):  # [(t2,f), (t2,o)]
        Z = np.zeros((128, 128), dtype=np.float32)
        Z[0:F, 0:O] = Wm
        Z[F:128, O:128] = Wm
        return Z.astype(ml_dtypes.float8_e4m3fn)

    w1z = blockdiag2(W1)
    w0z = blockdiag2(W0)
    wz4 = np.zeros((128, 2, 256), dtype=np.float32)
    for t1 in range(2):
        for t2 in range(2):
            u = 2 * t1 + t2
            wz4[t2 * F:(t2 + 1) * F, t1, u * O:(u + 1) * O] = W2
    wz4 = wz4.astype(ml_dtypes.float8_e4m3fn)

    x64 = x_b.astype(np.float64)
    a64 = adj_b.astype(np.float64)
    sx = x64.sum(axis=1)                                   # [F, T]
    v1 = 0.5 * (sx.T @ W1.astype(np.float64)).reshape(C)   # (t, o)
    v2 = 0.5 * (sx.T @ W2.astype(np.float64)).reshape(C)
    qc = a64.sum(axis=0) - 0.5 * N
    rc = a64.sum(axis=1) - 0.5 * N
    xqc = np.einsum("m,fmt->ft", qc, x64)
    swc = (xqc.T @ W2.astype(np.float64)).reshape(C)
    bc = (0.5 * N) * v2 + 0.5 * swc
    vrow = np.tile(
        np.concatenate([v1, v2, bc]).astype(np.float32)[None, :], (128, 1)
    )
    rcol = np.ascontiguousarray(rc.reshape(NB, 128).T.astype(np.float32))
    # v1col[(t2,o), (q,t1)] = v1[4q+2t1+t2, o]
    v1r = v1.reshape(NQ, 2, 2, O)                          # [q, t1, t2, o]
    v1col = np.ascontiguousarray(
        v1r.transpose(2, 3, 0, 1).reshape(128, 2 * NQ).astype(np.float32)
    )
    return {
        "xq": xq, "xtq": xtq, "adjm": adjm, "adjc": adjc,
        "w1z": w1z, "w0z": w0z, "wz4": wz4,
        "v1col": v1col, "vrow": vrow, "rcol": rcol,
    }


_NC = None
LAST_RESULTS = None


def kernel(x, adj, W0, b0, W1, b1, W2, b2):
    global _NC, LAST_RESULTS
    x = np.asarray(x, dtype=np.float32)
    adj = np.asarray(adj, dtype=np.float32)
    W0 = np.asarray(W0, dtype=np.float32)
    W1 = np.asarray(W1, dtype=np.float32)
    W2 = np.asarray(W2, dtype=np.float32)
    B = x.shape[0]
    assert B == 8 and x.shape == (8, F, N, T) and adj.shape == (8, N, N)

    if _NC is None:
        _NC = build_nc()

    in_maps = [host_prep(x[b], adj[b], W0, W1, W2) for b in range(B)]
    nwarm = int(os.environ.get("KERNEL_WARMUP_RUNS", "0"))
    for _ in range(nwarm):
        run_bass_kernel_spmd(_NC, in_maps, core_ids=list(range(8)))
    res = run_bass_kernel_spmd(_NC, in_maps, core_ids=list(range(8)))
    LAST_RESULTS = res

    out = np.empty((B, 3 * O, N, T), dtype=np.float32)
    for b in range(B):
        r = res.results[b]
        # feature-major [ (q,t1), (t2,o), n ] -> [o, n, t]
        def unT(a):
            v = a.astype(np.float32).reshape(NQ, 2, 2, O, N)  # q t1 t2 o n
            return v.transpose(3, 4, 0, 1, 2).reshape(O, N, T)
        out[b, 0:O] = unT(r["p0t"])
        out[b, O:2 * O] = unT(r["z1t"])
        out[b, 2 * O:3 * O] = (
            r["z2"].astype(np.float32).reshape(N, T, O).transpose(2, 0, 1)
        )
    del b0, b1, b2
    return out


# revision 22
# speedup vs baseline: 1.1823x; 1.1823x over previous
"""MixHop Trainium2 kernel v3 — diffuse raw X, then apply W.

Identity used: a@(X@Wz) == (a@X)@Wz (diffusion acts on nodes, W on
features), so the 1536-col pass A of v2 becomes a 768-col diffusion of
the raw feature block X, halving the dominant matmul:
    Y1  = ac@Xq                   (pass A'', feature-major psum out)
    z1  = lrelu(Y1q@W1z + v1)     (big 512-col matmuls, bias per-partition)
    wc  = Y1q@W2z4 (node-major)   (DoubleRow, 48 matmuls)
    z2  = lrelu(ac@wc + B + rc x v2)   (pass B, node-major, as v2)
    p0  = lrelu(Xq@W0z)           (from host feature-major X)
All rank-1 common-mode corrections stay host-exact (centered adjacency).
"""

import os
import sys

if "/opt/trn_rl_repo" not in sys.path:
    sys.path.insert(0, "/opt/trn_rl_repo")

import ml_dtypes
import numpy as np

import concourse.bass as bass
import concourse.tile as tile
from concourse import bacc, mybir
from concourse.bass_utils import run_bass_kernel_spmd

F = 64
O = 64
N = 2048
T = 12
NB = N // 128
KB = N // 256
C = O * T          # 768
XC = F * T         # 768 X columns, (t, f)
NQ = 3             # t-quads

F32 = mybir.dt.float32
BF16 = mybir.dt.bfloat16
FP8 = mybir.dt.float8e4
DR = mybir.MatmulPerfMode.DoubleRow
ACT_FUNC = mybir.ActivationFunctionType.Lrelu
LEAKY_SLOPE = 0.01


def build_nc(num_devices=8):
    nc = bacc.Bacc("TRN2", target_bir_lowering=False, debug=False,
                   num_devices=num_devices)

    # ---- DRAM I/O ----------------------------------------------------------
    # node-major X pairs (pass A'' stationary): [kb, p, j, (t,f)]
    xq_d = nc.dram_tensor("xq", [KB, 128, 2, XC], FP8, kind="ExternalInput").ap()
    # feature-major X quads (p0 moving): [q, (t2,f), t1, n]
    xtq_d = nc.dram_tensor("xtq", [NQ, 128, 2, N], FP8, kind="ExternalInput").ap()
    # moving adjacency for A'': [kb, p(m_low), j, n] = ac[n, m]
    adjm_d = nc.dram_tensor("adjm", [KB, 128, 2, N], FP8, kind="ExternalInput").ap()
    # stationary adjacency for pass B: [nb, p, kb, j, nl] = ac[nb*128+nl, m]
    adjc_d = nc.dram_tensor("adjc", [NB, 128, KB, 2, 128], FP8,
                            kind="ExternalInput").ap()
    # weight blocks (fp8): w1z/w0z [(t2,f), (t2,o)]; wz4 [(t2,f), t1, (u,o)]
    w1z_d = nc.dram_tensor("w1z", [128, 128], FP8, kind="ExternalInput").ap()
    w0z_d = nc.dram_tensor("w0z", [128, 128], FP8, kind="ExternalInput").ap()
    wz4_d = nc.dram_tensor("wz4", [128, 2, 256], FP8, kind="ExternalInput").ap()
    # v1col [ (t2,o), (q,t1) ]; vrow [v1|v2|B] replicated; rcol
    v1col_d = nc.dram_tensor("v1col", [128, 2 * NQ], F32, kind="ExternalInput").ap()
    vrow_d = nc.dram_tensor("vrow", [128, 3 * C], F32, kind="ExternalInput").ap()
    rcol_d = nc.dram_tensor("rcol", [128, NB], F32, kind="ExternalInput").ap()

    # outputs: p0/z1 feature-major [ (q,t1), (t2,o), n ]; z2 node-major
    p0t_d = nc.dram_tensor("p0t", [2 * NQ, 128, N], BF16, kind="ExternalOutput").ap()
    z1t_d = nc.dram_tensor("z1t", [2 * NQ, 128, N], BF16, kind="ExternalOutput").ap()
    z2_d = nc.dram_tensor("z2", [N, C], BF16, kind="ExternalOutput").ap()

    lrelu = ACT_FUNC
    add = mybir.AluOpType.add
    mult = mybir.AluOpType.mult
    amax = mybir.AluOpType.max

    with tile.TileContext(nc) as tc:
        with (
            tc.tile_pool(name="consts", bufs=1) as consts,
            tc.tile_pool(name="xq", bufs=1) as xqp,
            tc.tile_pool(name="adjm", bufs=1) as adjmp,
            tc.tile_pool(name="adjc", bufs=1) as adjcp,
            tc.tile_pool(name="y1t", bufs=1) as y1tp,
            tc.tile_pool(name="xtq", bufs=1) as xtqp,
            tc.tile_pool(name="wq", bufs=KB) as wqp,
            tc.tile_pool(name="zst", bufs=6) as zstp,
            tc.tile_pool(name="tmp", bufs=4) as tmpp,
        ):
            # ---- loads (sync ring), in need-order -------------------------
            xq_t = xqp.tile([128, KB * 2 * XC], FP8, name="xqall")
            nc.sync.dma_start(
                out=xq_t[:].rearrange("p (kb r) -> p kb r", kb=KB),
                in_=xq_d.rearrange("kb p j c -> p kb (j c)"),
            )
            adjm_ch = []
            for cix in range(4):
                amc = adjmp.tile([128, 2 * 2 * N], FP8, tag=f"adjm{cix}",
                                 name=f"adjm{cix}")
                nc.sync.dma_start(
                    out=amc[:].rearrange("p (kb r) -> p kb r", kb=2),
                    in_=adjm_d[2 * cix:2 * cix + 2].rearrange(
                        "kb p j n -> p kb (j n)"
                    ),
                )
                adjm_ch.append(amc)
            w1z_t = consts.tile([128, 128], FP8, tag="w1z")
            nc.sync.dma_start(out=w1z_t[:], in_=w1z_d)
            w0z_t = consts.tile([128, 128], FP8, tag="w0z")
            nc.sync.dma_start(out=w0z_t[:], in_=w0z_d)
            wz4_t = consts.tile([128, 512], FP8, tag="wz4")
            nc.sync.dma_start(
                out=wz4_t[:].rearrange("p (j c) -> p j c", j=2), in_=wz4_d
            )
            v1c_t = consts.tile([128, 2 * NQ], F32, tag="v1c")
            nc.sync.dma_start(out=v1c_t[:], in_=v1col_d)
            vrow_t = consts.tile([128, 3 * C], F32, tag="vrow")
            nc.sync.dma_start(out=vrow_t[:], in_=vrow_d)
            rc_t = consts.tile([128, NB], F32, tag="rc")
            nc.sync.dma_start(out=rc_t[:], in_=rcol_d)
            xtq_t = xtqp.tile([128, NQ * 2 * N], FP8, name="xtqall")
            nc.sync.dma_start(
                out=xtq_t[:].rearrange("p (q r) -> p q r", q=NQ),
                in_=xtq_d.rearrange("q p j n -> p q (j n)"),
            )
            adjc_t = adjcp.tile([128, NB * N], FP8, name="adjcall")
            nc.sync.dma_start(
                out=adjc_t[:].rearrange("p (nb r) -> p nb r", nb=NB),
                in_=adjc_d.rearrange("nb p a b c -> p nb (a b c)"),
            )

            def xq_sl(kb, ch):  # stationary [128, 2, 128] for A'' chunk ch
                return xq_t[:].rearrange(
                    "p (kb j c) -> p kb j c", kb=KB, j=2
                )[:, kb, :, ch * 128:(ch + 1) * 128]

            def adjm_sl(kb, ns):  # moving [128, 2, 256]
                return adjm_ch[kb // 2][:].rearrange(
                    "p (kb j n) -> p kb j n", kb=2, j=2
                )[:, kb % 2, :, ns * 256:(ns + 1) * 256]

            def adjc_sl(nb):  # pass-B stationary [128, kb, j, nl]
                return adjc_t[:, nb * N:(nb + 1) * N].rearrange(
                    "p (kb j nl) -> p kb j nl", kb=KB, j=2
                )

            # ---- pass A'': Y1T[ch] = (ac@X)^T chunk, feature-major --------
            # psum [c2=128, n=2048] (4 banks) x 2 bufs; 6 chunks (q, t1).
            y1 = []
            for q in range(NQ):
                y1.append(
                    y1tp.tile([128, 2 * N], FP8, tag=f"y1{q}", name=f"y1q{q}")
                )
            # kb-outer over chunk pairs (one t-quad per sweep): the first
            # sweep consumes adjm slabs as they stream in instead of
            # waiting for the full 4.2MB load.
            with tc.tile_pool(name="psA", bufs=2, space="PSUM") as psA:
                for q in range(NQ):
                    pzs = [
                        psA.tile([128, N], F32, tag="A", name=f"pzA{q}_{t1}")
                        for t1 in range(2)
                    ]
                    for kb in range(KB):
                        for t1 in range(2):
                            lhsT = xq_sl(kb, 2 * q + t1)
                            for ns in range(KB):
                                nc.tensor.matmul(
                                    pzs[t1][:, ns * 256:(ns + 1) * 256],
                                    lhsT,
                                    adjm_sl(kb, ns),
                                    start=(kb == 0 and ns % 2 == 0),
                                    stop=(kb == KB - 1 and ns % 2 == 1),
                                    perf_mode=DR,
                                )
                    for t1 in range(2):
                        dst = y1[q][:].rearrange("p (j n) -> p j n", j=2)[:, t1]
                        nc.vector.tensor_copy(dst[:, 0:N // 2],
                                              pzs[t1][:, 0:N // 2])
                        nc.scalar.activation(
                            dst[:, N // 2:N], pzs[t1][:, N // 2:N],
                            mybir.ActivationFunctionType.Copy,
                        )

            # ---- wc-Wmult: wc = Y1q@W2z4, node-major (for pass B) -----
            # two nb per psum bank, one batched 3D fp8 copy per nb-pair.
            wq = []
            for k in range(KB):
                wq.append(
                    wqp.tile([128, 2 * C], FP8, tag="wq", name=f"wq{k}")
                )
            with tc.tile_pool(name="psW", bufs=3, space="PSUM") as psW:
                for q in range(NQ):
                    y1v = y1[q][:].rearrange("p (j n) -> p j n", j=2)
                    for k in range(KB):
                        pw = psW.tile([128, 512], F32, tag="W")
                        for u in range(2):
                            nb = 2 * k + u
                            nc.tensor.matmul(
                                pw[:, u * 256:(u + 1) * 256],
                                y1v[:, :, nb * 128:(nb + 1) * 128],
                                wz4_t[:].rearrange("p (j c) -> p j c", j=2),
                                start=(u == 0), stop=(u == 1),
                                perf_mode=DR,
                            )
                        dstw = wq[k][:].rearrange("p (j c) -> p j c", j=2)[
                            :, :, q * 256:(q + 1) * 256
                        ]
                        srcw = pw[:].rearrange("p (u c) -> p u c", u=2)
                        if k % 2 == 0:
                            nc.vector.tensor_copy(dstw, srcw)
                        else:
                            nc.scalar.activation(
                                dstw, srcw, mybir.ActivationFunctionType.Copy
                            )

                # ---- pass B with z1/p0 Wmults interleaved (3 per nb) ------
                # z1 drain: one ACT (bias=v1col + lrelu); p0 drain: one DVE
                # scalar_tensor_tensor leaky (max(0.01x, x)).
                wjobs = [("z1", ch, ns) for ch in range(2 * NQ) for ns in range(4)]
                wjobs += [("p0", ch, ns) for ch in range(2 * NQ) for ns in range(4)]

                def wmult_job(kind, ch, ns):
                    q, t1 = ch // 2, ch % 2
                    pw = psW.tile([128, 512], F32, tag="W")
                    if kind == "z1":
                        mv = y1[q][:].rearrange("p (j n) -> p j n", j=2)[
                            :, t1, ns * 512:(ns + 1) * 512
                        ]
                        nc.tensor.matmul(pw[:], w1z_t[:], mv,
                                         start=True, stop=True)
                        zt = zstp.tile([128, 512], BF16, tag="zst")
                        nc.scalar.activation(
                            zt[:], pw[:], lrelu, alpha=0.01,
                            bias=v1c_t[:, ch:ch + 1],
                        )
                        nc.sync.dma_start(
                            out=z1t_d[ch, :, ns * 512:(ns + 1) * 512], in_=zt[:]
                        )
                    else:
                        mv = xtq_t[:].rearrange(
                            "p (q j n) -> p q j n", q=NQ, j=2
                        )[:, q, t1, ns * 512:(ns + 1) * 512]
                        nc.tensor.matmul(pw[:], w0z_t[:], mv,
                                         start=True, stop=True)
                        zt = zstp.tile([128, 512], BF16, tag="zst")
                        nc.scalar.activation(zt[:], pw[:], lrelu, alpha=0.01)
                        nc.sync.dma_start(
                            out=p0t_d[ch, :, ns * 512:(ns + 1) * 512], in_=zt[:]
                        )

                with tc.tile_pool(name="psB", bufs=2, space="PSUM") as psB:
                    for nb in range(NB):
                        pz = psB.tile([128, C], F32, tag="B")
                        sv = adjc_sl(nb)
                        for kb in range(KB):
                            wv = wq[kb][:].rearrange("p (j c) -> p j c", j=2)
                            for s in range(3):
                                nc.tensor.matmul(
                                    pz[:, s * 256:(s + 1) * 256],
                                    sv[:, kb],
                                    wv[:, :, s * 256:(s + 1) * 256],
                                    start=(kb == 0 and s % 2 == 0),
                                    stop=(kb == KB - 1 and s >= 1),
                                    perf_mode=DR,
                                )
                        for _ in range(3):
                            if wjobs:
                                wmult_job(*wjobs.pop(0))
                        s2 = tmpp.tile([128, C], F32, tag="tmp")
                        nc.vector.scalar_tensor_tensor(
                            s2[:], vrow_t[:, C:2 * C], rc_t[:, nb:nb + 1],
                            vrow_t[:, 2 * C:3 * C], mult, add,
                        )
                        tmp = tmpp.tile([128, C], F32, tag="tmp")
                        nc.vector.tensor_tensor(tmp[:], pz[:], s2[:], add)
                        zt = zstp.tile([128, C], BF16, tag="zstB")
                        nc.vector.scalar_tensor_tensor(
                            zt[:], tmp[:], LEAKY_SLOPE, tmp[:], mult, amax
                        )
                        nc.sync.dma_start(
                            out=z2_d[nb * 128:(nb + 1) * 128, :], in_=zt[:]
                        )
                    while wjobs:
                        wmult_job(*wjobs.pop(0))

    nc.finalize()
    return nc


def host_prep(x_b, adj_b, W0, W1, W2):
    Xf = np.ascontiguousarray(x_b.transpose(1, 2, 0)).reshape(N, XC)  # [n,(t,f)]
    X8 = Xf.astype(ml_dtypes.float8_e4m3fn)
    xq = np.ascontiguousarray(
        X8.reshape(KB, 2, 128, XC).transpose(0, 2, 1, 3)
    )
    xtq = np.ascontiguousarray(
        X8.reshape(N, NQ, 2, 2, F).transpose(1, 3, 4, 2, 0).reshape(NQ, 128, 2, N)
    )
    ac = (adj_b - np.float32(0.5))
    ac8 = ac.astype(ml_dtypes.float8_e4m3fn)
    acT = np.ascontiguousarray(ac8.T)  # [m, n]
    adjm = np.ascontiguousarray(
        acT.reshape(KB, 2, 128, N).transpose(0, 2, 1, 3)
    )
    adjc = np.ascontiguousarray(
        ac8.reshape(NB, 128, KB, 2, 128).transpose(0, 4, 2, 3, 1)
    )

    def blockdiag2(Wm):  # [(t2,f), (t2,o)]
        Z = np.zeros((128, 128), dtype=np.float32)
        Z[0:F, 0:O] = Wm
        Z[F:128, O:128] = Wm
        return Z.astype(ml_dtypes.float8_e4m3fn)

    w1z = blockdiag2(W1)
    w0z = blockdiag2(W0)
    wz4 = np.zeros((128, 2, 256), dtype=np.float32)
    for t1 in range(2):
        for t2 in range(2):
            u = 2 * t1 + t2
            wz4[t2 * F:(t2 + 1) * F, t1, u * O:(u + 1) * O] = W2
    wz4 = wz4.astype(ml_dtypes.float8_e4m3fn)

    x64 = x_b.astype(np.float64)
    a64 = adj_b.astype(np.float64)
    sx = x64.sum(axis=1)                                   # [F, T]
    v1 = 0.5 * (sx.T @ W1.astype(np.float64)).reshape(C)   # (t, o)
    v2 = 0.5 * (sx.T @ W2.astype(np.float64)).reshape(C)
    qc = a64.sum(axis=0) - 0.5 * N
    rc = a64.sum(axis=1) - 0.5 * N
    xqc = np.einsum("m,fmt->ft", qc, x64)
    swc = (xqc.T @ W2.astype(np.float64)).reshape(C)
    bc = (0.5 * N) * v2 + 0.5 * swc
    vrow = np.tile(
        np.concatenate([v1, v2, bc]).astype(np.float32)[None, :], (128, 1)
    )
    rcol = np.ascontiguousarray(rc.reshape(NB, 128).T.astype(np.float32))
    # v1col[(t2,o), (q,t1)] = v1[4q+2t1+t2, o]
    v1r = v1.reshape(NQ, 2, 2, O)                          # [q, t1, t2, o]
    v1col = np.ascontiguousarray(
        v1r.transpose(2, 3, 0, 1).reshape(128, 2 * NQ).astype(np.float32)
    )
    return {
        "xq": xq, "xtq": xtq, "adjm": adjm, "adjc": adjc,
        "w1z": w1z, "w0z": w0z, "wz4": wz4,
        "v1col": v1col, "vrow": vrow, "rcol": rcol,
    }


_NC = None
LAST_RESULTS = None


def kernel(x, adj, W0, b0, W1, b1, W2, b2):
    global _NC, LAST_RESULTS
    x = np.asarray(x, dtype=np.float32)
    adj = np.asarray(adj, dtype=np.float32)
    W0 = np.asarray(W0, dtype=np.float32)
    W1 = np.asarray(W1, dtype=np.float32)
    W2 = np.asarray(W2, dtype=np.float32)
    B = x.shape[0]
    assert B == 8 and x.shape == (8, F, N, T) and adj.shape == (8, N, N)

    if _NC is None:
        _NC = build_nc()

    in_maps = [host_prep(x[b], adj[b], W0, W1, W2) for b in range(B)]
    nwarm = int(os.environ.get("KERNEL_WARMUP_RUNS", "0"))
    for _ in range(nwarm):
        run_bass_kernel_spmd(_NC, in_maps, core_ids=list(range(8)))
    res = run_bass_kernel_spmd(_NC, in_maps, core_ids=list(range(8)))
    LAST_RESULTS = res

    out = np.empty((B, 3 * O, N, T), dtype=np.float32)
    for b in range(B):
        r = res.results[b]
        # feature-major [ (q,t1), (t2,o), n ] -> [o, n, t]
        def unT(a):
            v = a.astype(np.float32).reshape(NQ, 2, 2, O, N)  # q t1 t2 o n
            return v.transpose(3, 4, 0, 1, 2).reshape(O, N, T)
        out[b, 0:O] = unT(r["p0t"])
        out[b, O:2 * O] = unT(r["z1t"])
        out[b, 2 * O:3 * O] = (
            r["z2"].astype(np.float32).reshape(N, T, O).transpose(2, 0, 1)
        )
    del b0, b1, b2
    return out


# revision 23
# speedup vs baseline: 1.1943x; 1.0102x over previous
"""MixHop Trainium2 kernel v3 — diffuse raw X, then apply W.

Identity used: a@(X@Wz) == (a@X)@Wz (diffusion acts on nodes, W on
features), so the 1536-col pass A of v2 becomes a 768-col diffusion of
the raw feature block X, halving the dominant matmul:
    Y1  = ac@Xq                   (pass A'', feature-major psum out)
    z1  = lrelu(Y1q@W1z + v1)     (big 512-col matmuls, bias per-partition)
    wc  = Y1q@W2z4 (node-major)   (DoubleRow, 48 matmuls)
    z2  = lrelu(ac@wc + B + rc x v2)   (pass B, node-major, as v2)
    p0  = lrelu(Xq@W0z)           (from host feature-major X)
All rank-1 common-mode corrections stay host-exact (centered adjacency).
"""

import os
import sys

if "/opt/trn_rl_repo" not in sys.path:
    sys.path.insert(0, "/opt/trn_rl_repo")

import ml_dtypes
import numpy as np

import concourse.bass as bass
import concourse.tile as tile
from concourse import bacc, mybir
from concourse.bass_utils import run_bass_kernel_spmd

F = 64
O = 64
N = 2048
T = 12
NB = N // 128
KB = N // 256
C = O * T          # 768
XC = F * T         # 768 X columns, (t, f)
NQ = 3             # t-quads

F32 = mybir.dt.float32
BF16 = mybir.dt.bfloat16
FP8 = mybir.dt.float8e4
DR = mybir.MatmulPerfMode.DoubleRow
ACT_FUNC = mybir.ActivationFunctionType.Lrelu
LEAKY_SLOPE = 0.01


def build_nc(num_devices=8):
    nc = bacc.Bacc("TRN2", target_bir_lowering=False, debug=False,
                   num_devices=num_devices)

    # ---- DRAM I/O ----------------------------------------------------------
    # node-major X pairs (pass A'' stationary): [kb, p, j, (t,f)]
    xq_d = nc.dram_tensor("xq", [KB, 128, 2, XC], FP8, kind="ExternalInput").ap()
    # feature-major X quads (p0 moving): [q, (t2,f), t1, n]
    xtq_d = nc.dram_tensor("xtq", [NQ, 128, 2, N], FP8, kind="ExternalInput").ap()
    # moving adjacency for A'': [kb, p(m_low), j, n] = ac[n, m]
    adjm_d = nc.dram_tensor("adjm", [KB, 128, 2, N], FP8, kind="ExternalInput").ap()
    # stationary adjacency for pass B: [nb, p, kb, j, nl] = ac[nb*128+nl, m]
    adjc_d = nc.dram_tensor("adjc", [NB, 128, KB, 2, 128], FP8,
                            kind="ExternalInput").ap()
    # weight blocks (fp8): w1z/w0z [(t2,f), (t2,o)]; wz4 [(t2,f), t1, (u,o)]
    w1z_d = nc.dram_tensor("w1z", [128, 128], FP8, kind="ExternalInput").ap()
    w0z_d = nc.dram_tensor("w0z", [128, 128], FP8, kind="ExternalInput").ap()
    wz4_d = nc.dram_tensor("wz4", [128, 2, 256], FP8, kind="ExternalInput").ap()
    # v1col [ (t2,o), (q,t1) ]; vrow [v1|v2|B] replicated; rcol
    v1col_d = nc.dram_tensor("v1col", [128, 2 * NQ], F32, kind="ExternalInput").ap()
    vrow_d = nc.dram_tensor("vrow", [128, 3 * C], F32, kind="ExternalInput").ap()
    rcol_d = nc.dram_tensor("rcol", [128, NB], F32, kind="ExternalInput").ap()

    # outputs: p0/z1 feature-major [ (q,t1), (t2,o), n ]; z2 node-major
    p0t_d = nc.dram_tensor("p0t", [2 * NQ, 128, N], BF16, kind="ExternalOutput").ap()
    z1t_d = nc.dram_tensor("z1t", [2 * NQ, 128, N], BF16, kind="ExternalOutput").ap()
    z2_d = nc.dram_tensor("z2", [N, C], BF16, kind="ExternalOutput").ap()

    lrelu = ACT_FUNC
    add = mybir.AluOpType.add
    mult = mybir.AluOpType.mult
    amax = mybir.AluOpType.max

    with tile.TileContext(nc) as tc:
        with (
            tc.tile_pool(name="consts", bufs=1) as consts,
            tc.tile_pool(name="xq", bufs=1) as xqp,
            tc.tile_pool(name="adjm", bufs=1) as adjmp,
            tc.tile_pool(name="adjc", bufs=1) as adjcp,
            tc.tile_pool(name="y1t", bufs=1) as y1tp,
            tc.tile_pool(name="xtq", bufs=1) as xtqp,
            tc.tile_pool(name="wq", bufs=KB) as wqp,
            tc.tile_pool(name="zst", bufs=6) as zstp,
            tc.tile_pool(name="tmp", bufs=4) as tmpp,
        ):
            # ---- loads (sync ring), in need-order -------------------------
            xq_ch = []
            adjm_ch = []
            for cix in range(4):
                xqc = xqp.tile([128, 2 * 2 * XC], FP8, tag=f"xq{cix}",
                               name=f"xqc{cix}")
                nc.sync.dma_start(
                    out=xqc[:].rearrange("p (kb r) -> p kb r", kb=2),
                    in_=xq_d[2 * cix:2 * cix + 2].rearrange(
                        "kb p j c -> p kb (j c)"
                    ),
                )
                xq_ch.append(xqc)
                amc = adjmp.tile([128, 2 * 2 * N], FP8, tag=f"adjm{cix}",
                                 name=f"adjm{cix}")
                nc.sync.dma_start(
                    out=amc[:].rearrange("p (kb r) -> p kb r", kb=2),
                    in_=adjm_d[2 * cix:2 * cix + 2].rearrange(
                        "kb p j n -> p kb (j n)"
                    ),
                )
                adjm_ch.append(amc)
            w1z_t = consts.tile([128, 128], FP8, tag="w1z")
            nc.sync.dma_start(out=w1z_t[:], in_=w1z_d)
            w0z_t = consts.tile([128, 128], FP8, tag="w0z")
            nc.sync.dma_start(out=w0z_t[:], in_=w0z_d)
            wz4_t = consts.tile([128, 512], FP8, tag="wz4")
            nc.sync.dma_start(
                out=wz4_t[:].rearrange("p (j c) -> p j c", j=2), in_=wz4_d
            )
            v1c_t = consts.tile([128, 2 * NQ], F32, tag="v1c")
            nc.sync.dma_start(out=v1c_t[:], in_=v1col_d)
            vrow_t = consts.tile([128, 3 * C], F32, tag="vrow")
            nc.sync.dma_start(out=vrow_t[:], in_=vrow_d)
            rc_t = consts.tile([128, NB], F32, tag="rc")
            nc.sync.dma_start(out=rc_t[:], in_=rcol_d)
            xtq_t = xtqp.tile([128, NQ * 2 * N], FP8, name="xtqall")
            nc.sync.dma_start(
                out=xtq_t[:].rearrange("p (q r) -> p q r", q=NQ),
                in_=xtq_d.rearrange("q p j n -> p q (j n)"),
            )
            adjc_t = adjcp.tile([128, NB * N], FP8, name="adjcall")
            nc.sync.dma_start(
                out=adjc_t[:].rearrange("p (nb r) -> p nb r", nb=NB),
                in_=adjc_d.rearrange("nb p a b c -> p nb (a b c)"),
            )

            def xq_sl(kb, ch):  # stationary [128, 2, 128] for A'' chunk ch
                return xq_ch[kb // 2][:].rearrange(
                    "p (kb j c) -> p kb j c", kb=2, j=2
                )[:, kb % 2, :, ch * 128:(ch + 1) * 128]

            def adjm_sl(kb, ns):  # moving [128, 2, 256]
                return adjm_ch[kb // 2][:].rearrange(
                    "p (kb j n) -> p kb j n", kb=2, j=2
                )[:, kb % 2, :, ns * 256:(ns + 1) * 256]

            def adjc_sl(nb):  # pass-B stationary [128, kb, j, nl]
                return adjc_t[:, nb * N:(nb + 1) * N].rearrange(
                    "p (kb j nl) -> p kb j nl", kb=KB, j=2
                )

            # ---- pass A'': Y1T[ch] = (ac@X)^T chunk, feature-major --------
            # psum [c2=128, n=2048] (4 banks) x 2 bufs; 6 chunks (q, t1).
            y1 = []
            for q in range(NQ):
                y1.append(
                    y1tp.tile([128, 2 * N], FP8, tag=f"y1{q}", name=f"y1q{q}")
                )
            # kb-outer over chunk pairs (one t-quad per sweep): the first
            # sweep consumes adjm slabs as they stream in instead of
            # waiting for the full 4.2MB load.
            with tc.tile_pool(name="psA", bufs=2, space="PSUM") as psA:
                for q in range(NQ):
                    pzs = [
                        psA.tile([128, N], F32, tag="A", name=f"pzA{q}_{t1}")
                        for t1 in range(2)
                    ]
                    for kb in range(KB):
                        for t1 in range(2):
                            lhsT = xq_sl(kb, 2 * q + t1)
                            for ns in range(KB):
                                nc.tensor.matmul(
                                    pzs[t1][:, ns * 256:(ns + 1) * 256],
                                    lhsT,
                                    adjm_sl(kb, ns),
                                    start=(kb == 0 and ns % 2 == 0),
                                    stop=(kb == KB - 1 and ns % 2 == 1),
                                    perf_mode=DR,
                                )
                    for t1 in range(2):
                        dst = y1[q][:].rearrange("p (j n) -> p j n", j=2)[:, t1]
                        qn = N // 4
                        for u in range(4):
                            sl = slice(u * qn, (u + 1) * qn)
                            if u % 2 == 0:
                                nc.vector.tensor_copy(dst[:, sl], pzs[t1][:, sl])
                            else:
                                nc.scalar.activation(
                                    dst[:, sl], pzs[t1][:, sl],
                                    mybir.ActivationFunctionType.Copy,
                                )

            # ---- wc-Wmult: wc = Y1q@W2z4, node-major (for pass B) -----
            # two nb per psum bank, one batched 3D fp8 copy per nb-pair.
            wq = []
            for k in range(KB):
                wq.append(
                    wqp.tile([128, 2 * C], FP8, tag="wq", name=f"wq{k}")
                )
            with tc.tile_pool(name="psW", bufs=3, space="PSUM") as psW:
                for q in range(NQ):
                    y1v = y1[q][:].rearrange("p (j n) -> p j n", j=2)
                    for k in range(KB):
                        pw = psW.tile([128, 512], F32, tag="W")
                        for u in range(2):
                            nb = 2 * k + u
                            nc.tensor.matmul(
                                pw[:, u * 256:(u + 1) * 256],
                                y1v[:, :, nb * 128:(nb + 1) * 128],
                                wz4_t[:].rearrange("p (j c) -> p j c", j=2),
                                start=(u == 0), stop=(u == 1),
                                perf_mode=DR,
                            )
                        dstw = wq[k][:].rearrange("p (j c) -> p j c", j=2)[
                            :, :, q * 256:(q + 1) * 256
                        ]
                        srcw = pw[:].rearrange("p (u c) -> p u c", u=2)
                        if k % 2 == 0:
                            nc.vector.tensor_copy(dstw, srcw)
                        else:
                            nc.scalar.activation(
                                dstw, srcw, mybir.ActivationFunctionType.Copy
                            )

                # ---- pass B with z1/p0 Wmults interleaved (3 per nb) ------
                # z1 drain: one ACT (bias=v1col + lrelu); p0 drain: one DVE
                # scalar_tensor_tensor leaky (max(0.01x, x)).
                wjobs = [("z1", ch, ns) for ch in range(2 * NQ) for ns in range(4)]
                wjobs += [("p0", ch, ns) for ch in range(2 * NQ) for ns in range(4)]

                def wmult_job(kind, ch, ns):
                    q, t1 = ch // 2, ch % 2
                    pw = psW.tile([128, 512], F32, tag="W")
                    if kind == "z1":
                        mv = y1[q][:].rearrange("p (j n) -> p j n", j=2)[
                            :, t1, ns * 512:(ns + 1) * 512
                        ]
                        nc.tensor.matmul(pw[:], w1z_t[:], mv,
                                         start=True, stop=True)
                        zt = zstp.tile([128, 512], BF16, tag="zst")
                        nc.scalar.activation(
                            zt[:], pw[:], lrelu, alpha=0.01,
                            bias=v1c_t[:, ch:ch + 1],
                        )
                        nc.sync.dma_start(
                            out=z1t_d[ch, :, ns * 512:(ns + 1) * 512], in_=zt[:]
                        )
                    else:
                        mv = xtq_t[:].rearrange(
                            "p (q j n) -> p q j n", q=NQ, j=2
                        )[:, q, t1, ns * 512:(ns + 1) * 512]
                        nc.tensor.matmul(pw[:], w0z_t[:], mv,
                                         start=True, stop=True)
                        zt = zstp.tile([128, 512], BF16, tag="zst")
                        nc.scalar.activation(zt[:], pw[:], lrelu, alpha=0.01)
                        nc.sync.dma_start(
                            out=p0t_d[ch, :, ns * 512:(ns + 1) * 512], in_=zt[:]
                        )

                with tc.tile_pool(name="psB", bufs=2, space="PSUM") as psB:
                    for nb in range(NB):
                        pz = psB.tile([128, C], F32, tag="B")
                        sv = adjc_sl(nb)
                        for kb in range(KB):
                            wv = wq[kb][:].rearrange("p (j c) -> p j c", j=2)
                            for s in range(3):
                                nc.tensor.matmul(
                                    pz[:, s * 256:(s + 1) * 256],
                                    sv[:, kb],
                                    wv[:, :, s * 256:(s + 1) * 256],
                                    start=(kb == 0 and s % 2 == 0),
                                    stop=(kb == KB - 1 and s >= 1),
                                    perf_mode=DR,
                                )
                        for _ in range(3):
                            if wjobs:
                                wmult_job(*wjobs.pop(0))
                        s2 = tmpp.tile([128, C], F32, tag="tmp")
                        nc.vector.scalar_tensor_tensor(
                            s2[:], vrow_t[:, C:2 * C], rc_t[:, nb:nb + 1],
                            vrow_t[:, 2 * C:3 * C], mult, add,
                        )
                        tmp = tmpp.tile([128, C], F32, tag="tmp")
                        nc.vector.tensor_tensor(tmp[:], pz[:], s2[:], add)
                        zt = zstp.tile([128, C], BF16, tag="zstB")
                        nc.vector.scalar_tensor_tensor(
                            zt[:], tmp[:], LEAKY_SLOPE, tmp[:], mult, amax
                        )
                        nc.sync.dma_start(
                            out=z2_d[nb * 128:(nb + 1) * 128, :], in_=zt[:]
                        )
                    while wjobs:
                        wmult_job(*wjobs.pop(0))

    nc.finalize()
    return nc


def host_prep(x_b, adj_b, W0, W1, W2):
    Xf = np.ascontiguousarray(x_b.transpose(1, 2, 0)).reshape(N, XC)  # [n,(t,f)]
    X8 = Xf.astype(ml_dtypes.float8_e4m3fn)
    xq = np.ascontiguousarray(
        X8.reshape(KB, 2, 128, XC).transpose(0, 2, 1, 3)
    )
    xtq = np.ascontiguousarray(
        X8.reshape(N, NQ, 2, 2, F).transpose(1, 3, 4, 2, 0).reshape(NQ, 128, 2, N)
    )
    ac = (adj_b - np.float32(0.5))
    ac8 = ac.astype(ml_dtypes.float8_e4m3fn)
    acT = np.ascontiguousarray(ac8.T)  # [m, n]
    adjm = np.ascontiguousarray(
        acT.reshape(KB, 2, 128, N).transpose(0, 2, 1, 3)
    )
    adjc = np.ascontiguousarray(
        ac8.reshape(NB, 128, KB, 2, 128).transpose(0, 4, 2, 3, 1)
    )

    def blockdiag2(Wm):  # [(t2,f), (t2,o)]
        Z = np.zeros((128, 128), dtype=np.float32)
        Z[0:F, 0:O] = Wm
        Z[F:128, O:128] = Wm
        return Z.astype(ml_dtypes.float8_e4m3fn)

    w1z = blockdiag2(W1)
    w0z = blockdiag2(W0)
    wz4 = np.zeros((128, 2, 256), dtype=np.float32)
    for t1 in range(2):
        for t2 in range(2):
            u = 2 * t1 + t2
            wz4[t2 * F:(t2 + 1) * F, t1, u * O:(u + 1) * O] = W2
    wz4 = wz4.astype(ml_dtypes.float8_e4m3fn)

    x64 = x_b.astype(np.float64)
    a64 = adj_b.astype(np.float64)
    sx = x64.sum(axis=1)                                   # [F, T]
    v1 = 0.5 * (sx.T @ W1.astype(np.float64)).reshape(C)   # (t, o)
    v2 = 0.5 * (sx.T @ W2.astype(np.float64)).reshape(C)
    qc = a64.sum(axis=0) - 0.5 * N
    rc = a64.sum(axis=1) - 0.5 * N
    xqc = np.einsum("m,fmt->ft", qc, x64)
    swc = (xqc.T @ W2.astype(np.float64)).reshape(C)
    bc = (0.5 * N) * v2 + 0.5 * swc
    vrow = np.tile(
        np.concatenate([v1, v2, bc]).astype(np.float32)[None, :], (128, 1)
    )
    rcol = np.ascontiguousarray(rc.reshape(NB, 128).T.astype(np.float32))
    # v1col[(t2,o), (q,t1)] = v1[4q+2t1+t2, o]
    v1r = v1.reshape(NQ, 2, 2, O)                          # [q, t1, t2, o]
    v1col = np.ascontiguousarray(
        v1r.transpose(2, 3, 0, 1).reshape(128, 2 * NQ).astype(np.float32)
    )
    return {
        "xq": xq, "xtq": xtq, "adjm": adjm, "adjc": adjc,
        "w1z": w1z, "w0z": w0z, "wz4": wz4,
        "v1col": v1col, "vrow": vrow, "rcol": rcol,
    }


_NC = None
LAST_RESULTS = None


def kernel(x, adj, W0, b0, W1, b1, W2, b2):
    global _NC, LAST_RESULTS
    x = np.asarray(x, dtype=np.float32)
    adj = np.asarray(adj, dtype=np.float32)
    W0 = np.asarray(W0, dtype=np.float32)
    W1 = np.asarray(W1, dtype=np.float32)
    W2 = np.asarray(W2, dtype=np.float32)
    B = x.shape[0]
    assert B == 8 and x.shape == (8, F, N, T) and adj.shape == (8, N, N)

    if _NC is None:
        _NC = build_nc()

    in_maps = [host_prep(x[b], adj[b], W0, W1, W2) for b in range(B)]
    nwarm = int(os.environ.get("KERNEL_WARMUP_RUNS", "0"))
    for _ in range(nwarm):
        run_bass_kernel_spmd(_NC, in_maps, core_ids=list(range(8)))
    res = run_bass_kernel_spmd(_NC, in_maps, core_ids=list(range(8)))
    LAST_RESULTS = res

    out = np.empty((B, 3 * O, N, T), dtype=np.float32)
    for b in range(B):
        r = res.results[b]
        # feature-major [ (q,t1), (t2,o), n ] -> [o, n, t]
        def unT(a):
            v = a.astype(np.float32).reshape(NQ, 2, 2, O, N)  # q t1 t2 o n
            return v.transpose(3, 4, 0, 1, 2).reshape(O, N, T)
        out[b, 0:O] = unT(r["p0t"])
        out[b, O:2 * O] = unT(r["z1t"])
        out[b, 2 * O:3 * O] = (
            r["z2"].astype(np.float32).reshape(N, T, O).transpose(2, 0, 1)
        )
    del b0, b1, b2
    return out


# revision 24
# speedup vs baseline: 1.2634x; 1.0578x over previous
"""MixHop Trainium2 kernel v3 — diffuse raw X, then apply W.

Identity used: a@(X@Wz) == (a@X)@Wz (diffusion acts on nodes, W on
features), so the 1536-col pass A of v2 becomes a 768-col diffusion of
the raw feature block X, halving the dominant matmul:
    Y1  = ac@Xq                   (pass A'', feature-major psum out)
    z1  = lrelu(Y1q@W1z + v1)     (big 512-col matmuls, bias per-partition)
    wc  = Y1q@W2z4 (node-major)   (DoubleRow, 48 matmuls)
    z2  = lrelu(ac@wc + B + rc x v2)   (pass B, node-major, as v2)
    p0  = lrelu(Xq@W0z)           (from host feature-major X)
All rank-1 common-mode corrections stay host-exact (centered adjacency).
"""

import os
import sys

if "/opt/trn_rl_repo" not in sys.path:
    sys.path.insert(0, "/opt/trn_rl_repo")

import ml_dtypes
import numpy as np

import concourse.bass as bass
import concourse.tile as tile
from concourse import bacc, mybir
from concourse.bass_utils import run_bass_kernel_spmd

F = 64
O = 64
N = 2048
T = 12
NB = N // 128
KB = N // 256
C = O * T          # 768
XC = F * T         # 768 X columns, (t, f)
NQ = 3             # t-quads

F32 = mybir.dt.float32
BF16 = mybir.dt.bfloat16
FP8 = mybir.dt.float8e4
DR = mybir.MatmulPerfMode.DoubleRow
ACT_FUNC = mybir.ActivationFunctionType.Lrelu
LEAKY_SLOPE = 0.01


def build_nc(num_devices=8):
    nc = bacc.Bacc("TRN2", target_bir_lowering=False, debug=False,
                   num_devices=num_devices)

    # ---- DRAM I/O ----------------------------------------------------------
    # node-major X pairs (pass A'' stationary): [kb, p, j, (t,f)]
    xq_d = nc.dram_tensor("xq", [KB, 128, 2, XC], FP8, kind="ExternalInput").ap()
    # feature-major X quads (p0 moving): [q, (t2,f), t1, n]
    xtq_d = nc.dram_tensor("xtq", [NQ, 128, 2, N], FP8, kind="ExternalInput").ap()
    # moving adjacency for A'': [kb, p(m_low), j, n] = ac[n, m]
    adjm_d = nc.dram_tensor("adjm", [KB, 128, 2, N], FP8, kind="ExternalInput").ap()
    # stationary adjacency for pass B: [nb, p, kb, j, nl] = ac[nb*128+nl, m]
    adjc_d = nc.dram_tensor("adjc", [NB, 128, KB, 2, 128], FP8,
                            kind="ExternalInput").ap()
    # weight blocks (fp8): w1z/w0z [(t2,f), (t2,o)]; wz4 [(t2,f), t1, (u,o)]
    w1z_d = nc.dram_tensor("w1z", [128, 128], FP8, kind="ExternalInput").ap()
    w0z_d = nc.dram_tensor("w0z", [128, 128], FP8, kind="ExternalInput").ap()
    wz4_d = nc.dram_tensor("wz4", [128, 2, 256], FP8, kind="ExternalInput").ap()
    # v1col [ (t2,o), (q,t1) ]; vrow [v1|v2|B] replicated; rcol
    v1col_d = nc.dram_tensor("v1col", [128, 2 * NQ], F32, kind="ExternalInput").ap()
    vrow_d = nc.dram_tensor("vrow", [128, 3 * C], F32, kind="ExternalInput").ap()
    rcol_d = nc.dram_tensor("rcol", [128, NB], F32, kind="ExternalInput").ap()

    # outputs: p0/z1 feature-major [ (q,t1), (t2,o), n ]; z2 node-major
    p0t_d = nc.dram_tensor("p0t", [2 * NQ, 128, N], BF16, kind="ExternalOutput").ap()
    z1t_d = nc.dram_tensor("z1t", [2 * NQ, 128, N], BF16, kind="ExternalOutput").ap()
    z2_d = nc.dram_tensor("z2", [N, C], BF16, kind="ExternalOutput").ap()

    lrelu = ACT_FUNC
    add = mybir.AluOpType.add
    mult = mybir.AluOpType.mult
    amax = mybir.AluOpType.max

    with tile.TileContext(nc) as tc:
        with (
            tc.tile_pool(name="consts", bufs=1) as consts,
            tc.tile_pool(name="xq", bufs=1) as xqp,
            tc.tile_pool(name="adjm", bufs=1) as adjmp,
            tc.tile_pool(name="adjc", bufs=1) as adjcp,
            tc.tile_pool(name="y1t", bufs=1) as y1tp,
            tc.tile_pool(name="xtq", bufs=1) as xtqp,
            tc.tile_pool(name="wq", bufs=KB) as wqp,
            tc.tile_pool(name="zst", bufs=6) as zstp,
            tc.tile_pool(name="tmp", bufs=4) as tmpp,
        ):
            # ---- loads (sync ring), in need-order -------------------------
            xq_ch = []
            adjm_ch = []
            for cix in range(4):
                xqc = xqp.tile([128, 2 * 2 * XC], FP8, tag=f"xq{cix}",
                               name=f"xqc{cix}")
                nc.sync.dma_start(
                    out=xqc[:].rearrange("p (kb r) -> p kb r", kb=2),
                    in_=xq_d[2 * cix:2 * cix + 2].rearrange(
                        "kb p j c -> p kb (j c)"
                    ),
                )
                xq_ch.append(xqc)
                amc = adjmp.tile([128, 2 * 2 * N], FP8, tag=f"adjm{cix}",
                                 name=f"adjm{cix}")
                nc.sync.dma_start(
                    out=amc[:].rearrange("p (kb r) -> p kb r", kb=2),
                    in_=adjm_d[2 * cix:2 * cix + 2].rearrange(
                        "kb p j n -> p kb (j n)"
                    ),
                )
                adjm_ch.append(amc)
            w1z_t = consts.tile([128, 128], FP8, tag="w1z")
            nc.sync.dma_start(out=w1z_t[:], in_=w1z_d)
            w0z_t = consts.tile([128, 128], FP8, tag="w0z")
            nc.sync.dma_start(out=w0z_t[:], in_=w0z_d)
            wz4_t = consts.tile([128, 512], FP8, tag="wz4")
            nc.sync.dma_start(
                out=wz4_t[:].rearrange("p (j c) -> p j c", j=2), in_=wz4_d
            )
            v1c_t = consts.tile([128, 2 * NQ], F32, tag="v1c")
            nc.sync.dma_start(out=v1c_t[:], in_=v1col_d)
            vrow_t = consts.tile([128, 3 * C], F32, tag="vrow")
            nc.sync.dma_start(out=vrow_t[:], in_=vrow_d)
            rc_t = consts.tile([128, NB], F32, tag="rc")
            nc.sync.dma_start(out=rc_t[:], in_=rcol_d)
            xtq_t = xtqp.tile([128, NQ * 2 * N], FP8, name="xtqall")
            nc.sync.dma_start(
                out=xtq_t[:].rearrange("p (q r) -> p q r", q=NQ),
                in_=xtq_d.rearrange("q p j n -> p q (j n)"),
            )
            adjc_t = adjcp.tile([128, NB * N], FP8, name="adjcall")
            nc.sync.dma_start(
                out=adjc_t[:].rearrange("p (nb r) -> p nb r", nb=NB),
                in_=adjc_d.rearrange("nb p a b c -> p nb (a b c)"),
            )

            def xq_sl(kb, ch):  # stationary [128, 2, 128] for A'' chunk ch
                return xq_ch[kb // 2][:].rearrange(
                    "p (kb j c) -> p kb j c", kb=2, j=2
                )[:, kb % 2, :, ch * 128:(ch + 1) * 128]

            def adjm_sl(kb, ns):  # moving [128, 2, 256]
                return adjm_ch[kb // 2][:].rearrange(
                    "p (kb j n) -> p kb j n", kb=2, j=2
                )[:, kb % 2, :, ns * 256:(ns + 1) * 256]

            def adjc_sl(nb):  # pass-B stationary [128, kb, j, nl]
                return adjc_t[:, nb * N:(nb + 1) * N].rearrange(
                    "p (kb j nl) -> p kb j nl", kb=KB, j=2
                )

            # ---- pass A'': Y1T[ch] = (ac@X)^T chunk, feature-major --------
            # psum [c2=128, n=2048] (4 banks) x 2 bufs; 6 chunks (q, t1).
            y1 = []
            for q in range(NQ):
                y1.append(
                    y1tp.tile([128, 2 * N], FP8, tag=f"y1{q}", name=f"y1q{q}")
                )
            # A'' in 12 half-units (ch, hf): [128, 1024] psum tiles
            # (2 banks, bufs=3) pipeline with no sweep-boundary bubbles;
            # 2 spare banks host the wc-Wmult pool so each quad's wc jobs
            # interleave into the next quad's compute.
            wq = []
            for k in range(KB):
                wq.append(
                    wqp.tile([128, 2 * C], FP8, tag="wq", name=f"wq{k}")
                )
            psA_cm = tc.tile_pool(name="psA", bufs=3, space="PSUM")
            psA = psA_cm.__enter__()
            psWa_cm = tc.tile_pool(name="psWa", bufs=2, space="PSUM")
            psWa = psWa_cm.__enter__()

            def wc_job(q, k):
                y1v = y1[q][:].rearrange("p (j n) -> p j n", j=2)
                pw = psWa.tile([128, 512], F32, tag="Wa", name=f"wc{q}_{k}")
                for u in range(2):
                    nb = 2 * k + u
                    nc.tensor.matmul(
                        pw[:, u * 256:(u + 1) * 256],
                        y1v[:, :, nb * 128:(nb + 1) * 128],
                        wz4_t[:].rearrange("p (j c) -> p j c", j=2),
                        start=(u == 0), stop=(u == 1),
                        perf_mode=DR,
                    )
                dstw = wq[k][:].rearrange("p (j c) -> p j c", j=2)[
                    :, :, q * 256:(q + 1) * 256
                ]
                srcw = pw[:].rearrange("p (u c) -> p u c", u=2)
                if k % 2 == 0:
                    nc.vector.tensor_copy(dstw, srcw)
                else:
                    nc.scalar.activation(
                        dstw, srcw, mybir.ActivationFunctionType.Copy
                    )

            HN = N // 2
            unit = 0
            for q in range(NQ):
                for t1 in range(2):
                    ch = 2 * q + t1
                    for hf in range(2):
                        pz = psA.tile([128, HN], F32, tag="A",
                                      name=f"pzA{ch}_{hf}")
                        for kb in range(KB):
                            lhsT = xq_sl(kb, ch)
                            for s in range(4):
                                ns = hf * 4 + s
                                nc.tensor.matmul(
                                    pz[:, s * 256:(s + 1) * 256],
                                    lhsT,
                                    adjm_sl(kb, ns),
                                    start=(kb == 0 and s % 2 == 0),
                                    stop=(kb == KB - 1 and s % 2 == 1),
                                    perf_mode=DR,
                                )
                        dst = y1[q][:].rearrange(
                            "p (j n) -> p j n", j=2
                        )[:, t1, hf * HN:(hf + 1) * HN]
                        if unit % 2 == 0:
                            nc.vector.tensor_copy(dst, pz[:])
                        else:
                            nc.scalar.activation(
                                dst, pz[:], mybir.ActivationFunctionType.Copy
                            )
                        unit += 1
                        # interleave previous quad's wc jobs (2 per unit)
                        if q >= 1:
                            for k in (2 * (2 * t1 + hf), 2 * (2 * t1 + hf) + 1):
                                wc_job(q - 1, k)
            for k in range(KB):
                wc_job(NQ - 1, k)
            psWa_cm.__exit__(None, None, None)
            psA_cm.__exit__(None, None, None)

            # ---- z1/p0 Wmults interleaved into pass B ---------------------
            with tc.tile_pool(name="psW", bufs=3, space="PSUM") as psW:
                # ---- pass B with z1/p0 Wmults interleaved (3 per nb) ------
                # z1 drain: one ACT (bias=v1col + lrelu); p0 drain: one DVE
                # scalar_tensor_tensor leaky (max(0.01x, x)).
                wjobs = [("z1", ch, ns) for ch in range(2 * NQ) for ns in range(4)]
                wjobs += [("p0", ch, ns) for ch in range(2 * NQ) for ns in range(4)]

                def wmult_job(kind, ch, ns):
                    q, t1 = ch // 2, ch % 2
                    pw = psW.tile([128, 512], F32, tag="W")
                    if kind == "z1":
                        mv = y1[q][:].rearrange("p (j n) -> p j n", j=2)[
                            :, t1, ns * 512:(ns + 1) * 512
                        ]
                        nc.tensor.matmul(pw[:], w1z_t[:], mv,
                                         start=True, stop=True)
                        zt = zstp.tile([128, 512], BF16, tag="zst")
                        nc.scalar.activation(
                            zt[:], pw[:], lrelu, alpha=0.01,
                            bias=v1c_t[:, ch:ch + 1],
                        )
                        nc.sync.dma_start(
                            out=z1t_d[ch, :, ns * 512:(ns + 1) * 512], in_=zt[:]
                        )
                    else:
                        mv = xtq_t[:].rearrange(
                            "p (q j n) -> p q j n", q=NQ, j=2
                        )[:, q, t1, ns * 512:(ns + 1) * 512]
                        nc.tensor.matmul(pw[:], w0z_t[:], mv,
                                         start=True, stop=True)
                        zt = zstp.tile([128, 512], BF16, tag="zst")
                        nc.scalar.activation(zt[:], pw[:], lrelu, alpha=0.01)
                        nc.sync.dma_start(
                            out=p0t_d[ch, :, ns * 512:(ns + 1) * 512], in_=zt[:]
                        )

                with tc.tile_pool(name="psB", bufs=2, space="PSUM") as psB:
                    for nb in range(NB):
                        pz = psB.tile([128, C], F32, tag="B")
                        sv = adjc_sl(nb)
                        for kb in range(KB):
                            wv = wq[kb][:].rearrange("p (j c) -> p j c", j=2)
                            for s in range(3):
                                nc.tensor.matmul(
                                    pz[:, s * 256:(s + 1) * 256],
                                    sv[:, kb],
                                    wv[:, :, s * 256:(s + 1) * 256],
                                    start=(kb == 0 and s % 2 == 0),
                                    stop=(kb == KB - 1 and s >= 1),
                                    perf_mode=DR,
                                )
                        for _ in range(3):
                            if wjobs:
                                wmult_job(*wjobs.pop(0))
                        s2 = tmpp.tile([128, C], F32, tag="tmp")
                        nc.vector.scalar_tensor_tensor(
                            s2[:], vrow_t[:, C:2 * C], rc_t[:, nb:nb + 1],
                            vrow_t[:, 2 * C:3 * C], mult, add,
                        )
                        tmp = tmpp.tile([128, C], F32, tag="tmp")
                        nc.vector.tensor_tensor(tmp[:], pz[:], s2[:], add)
                        zt = zstp.tile([128, C], BF16, tag="zstB")
                        nc.vector.scalar_tensor_tensor(
                            zt[:], tmp[:], LEAKY_SLOPE, tmp[:], mult, amax
                        )
                        nc.sync.dma_start(
                            out=z2_d[nb * 128:(nb + 1) * 128, :], in_=zt[:]
                        )
                    while wjobs:
                        wmult_job(*wjobs.pop(0))

    nc.finalize()
    return nc


def host_prep(x_b, adj_b, W0, W1, W2):
    Xf = np.ascontiguousarray(x_b.transpose(1, 2, 0)).reshape(N, XC)  # [n,(t,f)]
    X8 = Xf.astype(ml_dtypes.float8_e4m3fn)
    xq = np.ascontiguousarray(
        X8.reshape(KB, 2, 128, XC).transpose(0, 2, 1, 3)
    )
    xtq = np.ascontiguousarray(
        X8.reshape(N, NQ, 2, 2, F).transpose(1, 3, 4, 2, 0).reshape(NQ, 128, 2, N)
    )
    ac = (adj_b - np.float32(0.5))
    ac8 = ac.astype(ml_dtypes.float8_e4m3fn)
    acT = np.ascontiguousarray(ac8.T)  # [m, n]
    adjm = np.ascontiguousarray(
        acT.reshape(KB, 2, 128, N).transpose(0, 2, 1, 3)
    )
    adjc = np.ascontiguousarray(
        ac8.reshape(NB, 128, KB, 2, 128).transpose(0, 4, 2, 3, 1)
    )

    def blockdiag2(Wm):  # [(t2,f), (t2,o)]
        Z = np.zeros((128, 128), dtype=np.float32)
        Z[0:F, 0:O] = Wm
        Z[F:128, O:128] = Wm
        return Z.astype(ml_dtypes.float8_e4m3fn)

    w1z = blockdiag2(W1)
    w0z = blockdiag2(W0)
    wz4 = np.zeros((128, 2, 256), dtype=np.float32)
    for t1 in range(2):
        for t2 in range(2):
            u = 2 * t1 + t2
            wz4[t2 * F:(t2 + 1) * F, t1, u * O:(u + 1) * O] = W2
    wz4 = wz4.astype(ml_dtypes.float8_e4m3fn)

    x64 = x_b.astype(np.float64)
    a64 = adj_b.astype(np.float64)
    sx = x64.sum(axis=1)                                   # [F, T]
    v1 = 0.5 * (sx.T @ W1.astype(np.float64)).reshape(C)   # (t, o)
    v2 = 0.5 * (sx.T @ W2.astype(np.float64)).reshape(C)
    qc = a64.sum(axis=0) - 0.5 * N
    rc = a64.sum(axis=1) - 0.5 * N
    xqc = np.einsum("m,fmt->ft", qc, x64)
    swc = (xqc.T @ W2.astype(np.float64)).reshape(C)
    bc = (0.5 * N) * v2 + 0.5 * swc
    vrow = np.tile(
        np.concatenate([v1, v2, bc]).astype(np.float32)[None, :], (128, 1)
    )
    rcol = np.ascontiguousarray(rc.reshape(NB, 128).T.astype(np.float32))
    # v1col[(t2,o), (q,t1)] = v1[4q+2t1+t2, o]
    v1r = v1.reshape(NQ, 2, 2, O)                          # [q, t1, t2, o]
    v1col = np.ascontiguousarray(
        v1r.transpose(2, 3, 0, 1).reshape(128, 2 * NQ).astype(np.float32)
    )
    return {
        "xq": xq, "xtq": xtq, "adjm": adjm, "adjc": adjc,
        "w1z": w1z, "w0z": w0z, "wz4": wz4,
        "v1col": v1col, "vrow": vrow, "rcol": rcol,
    }


_NC = None
LAST_RESULTS = None


def kernel(x, adj, W0, b0, W1, b1, W2, b2):
    global _NC, LAST_RESULTS
    x = np.asarray(x, dtype=np.float32)
    adj = np.asarray(adj, dtype=np.float32)
    W0 = np.asarray(W0, dtype=np.float32)
    W1 = np.asarray(W1, dtype=np.float32)
    W2 = np.asarray(W2, dtype=np.float32)
    B = x.shape[0]
    assert B == 8 and x.shape == (8, F, N, T) and adj.shape == (8, N, N)

    if _NC is None:
        _NC = build_nc()

    in_maps = [host_prep(x[b], adj[b], W0, W1, W2) for b in range(B)]
    nwarm = int(os.environ.get("KERNEL_WARMUP_RUNS", "0"))
    for _ in range(nwarm):
        run_bass_kernel_spmd(_NC, in_maps, core_ids=list(range(8)))
    res = run_bass_kernel_spmd(_NC, in_maps, core_ids=list(range(8)))
    LAST_RESULTS = res

    out = np.empty((B, 3 * O, N, T), dtype=np.float32)
    for b in range(B):
        r = res.results[b]
        # feature-major [ (q,t1), (t2,o), n ] -> [o, n, t]
        def unT(a):
            v = a.astype(np.float32).reshape(NQ, 2, 2, O, N)  # q t1 t2 o n
            return v.transpose(3, 4, 0, 1, 2).reshape(O, N, T)
        out[b, 0:O] = unT(r["p0t"])
        out[b, O:2 * O] = unT(r["z1t"])
        out[b, 2 * O:3 * O] = (
            r["z2"].astype(np.float32).reshape(N, T, O).transpose(2, 0, 1)
        )
    del b0, b1, b2
    return out
